# revision 8
# baseline (speedup 1.0000x reference)
"""Bilateral denoiser (11x11 window, sigma=2) on 8 Trainium2 NeuronCores.

Math (per output pixel p, tap offset t=(dy,dx), d2 = dy^2+dx^2):
    w_t = exp(128*ln(dot(nrm_p, nrm_{p+t})) - |z_{p+t}-z_p|*min(1/(dz_p*c), 1e4)
              - d2/8)
        = clip(dot,0,1)^128 * exp(-|dz|/max(dz*sqrt(d2), 1e-4)) * exp(-d2/8)
    out = sum_t w_t * col_{p+t} / sum_t w_t     (center tap has w=1)

Sharding: H=1080 rows -> 10 "tile rows" of <=118 output rows, each computed
from a 128-row input tile (+-5 halo lives in the partition dim).  Every core
gets tile-row i full-width plus one 480-wide strip of tile-rows 8/9, so all
cores run an identical (SPMD) program on identically-shaped inputs.

Host side only pads/deinterleaves/slices (layout, no math); all arithmetic
runs on-device: DVE tensor ops + ScalarE Ln/Exp/Abs (one act table set).
"""
import math

import numpy as np

import concourse.bacc as bacc
import concourse.tile as tile
from concourse import mybir
from concourse.bass_utils import run_bass_kernel_spmd

F32 = mybir.dt.float32
AF = mybir.ActivationFunctionType
OP = mybir.AluOpType

RAD = 5
H, W = 1080, 1920
TILE_OUT = 118            # output rows per 128-partition tile
P0, P1 = RAD, RAD + TILE_OUT   # output partitions 5..123
RM_CLAMP = 1.0 / (128.0 * 1e-4)

# plane order inside the 8-plane tensors
# 0:3 = normal(xyz), 3 = z, 4:7 = color(rgb), 7 = dz
PLANE_PERM = [3, 4, 5, 6, 0, 1, 2, 7]  # from input channel order


def tap_classes(rad=RAD):
    cls = {}
    for dy in range(-rad, rad + 1):
        for dx in range(-rad, rad + 1):
            if dy == 0 and dx == 0:
                continue
            cls.setdefault(dy * dy + dx * dx, []).append((dy, dx))
    return sorted(cls.items())


def _dy_order():
    """dy iteration order; dx inner order groups +-dx to share rm planes."""
    return list(range(-RAD, RAD + 1))


def _emit_item(nc, pools, src_ap, dst_ap, n_in, classes, bias_tile, cls_idx):
    """One work item: input planes [8,128,n_in] -> output [118,3,n_in-10].

    Engine access patterns must start at partition 0 (quadrant rule), so the
    +-5-row halo shifts are materialized as partition-shifted DMA copies:
    sh_dy[p] = in_t[p+5+dy] for p in [0,118).
    """
    n_out = n_in - 2 * RAD
    inp, shp, t3p, t1p, accp, rmp = pools
    op = slice(0, TILE_OUT)            # all compute on partitions [0,118)
    of = slice(RAD, RAD + n_out)       # center free columns within n_in

    d2_set = {d2 for d2, _ in classes}

    in_t = inp.tile([128, 8, n_in], F32, tag="in", bufs=1)
    nc.sync.dma_start(out=in_t[:, :, :], in_=src_ap)

    # --- normalize normals in place: n *= (max(|n|^2,1e-20))^-0.5
    pr3 = t3p.tile([128, 3, n_in], F32, tag="p3", bufs=2)
    nc.vector.tensor_tensor(out=pr3[:, :, :], in0=in_t[:, 0:3, :],
                            in1=in_t[:, 0:3, :], op=OP.mult)
    nn = t1p.tile([128, n_in], F32, tag="s", bufs=2)
    nc.vector.tensor_tensor(out=nn[:, :], in0=pr3[:, 0, :], in1=pr3[:, 1, :],
                            op=OP.add)
    nc.vector.tensor_tensor(out=nn[:, :], in0=nn[:, :], in1=pr3[:, 2, :],
                            op=OP.add)
    nc.vector.tensor_scalar_max(out=nn[:, :], in0=nn[:, :], scalar1=1e-20)
    nc.scalar.activation(out=nn[:, :], in_=nn[:, :], func=AF.Ln)
    nc.scalar.activation(out=nn[:, :], in_=nn[:, :], func=AF.Exp, scale=-0.5)
    for c in range(3):
        nc.vector.tensor_tensor(out=in_t[:, c, :], in0=in_t[:, c, :],
                                in1=nn[:, :], op=OP.mult)

    # --- partition-aligned center copy (all 8 planes)
    ctr = shp.tile([128, 8, n_in], F32, tag="ctr", bufs=2)
    nc.sync.dma_start(out=ctr[op, :, :], in_=in_t[RAD:RAD + TILE_OUT, :, :])

    # --- rdz = 1/(128*dz) on center columns (inf when dz==0; min-clamped)
    rdz = t1p.tile([128, n_out], F32, tag="rdz", bufs=2)
    nc.scalar.activation(out=rdz[op, :], in_=ctr[op, 7, of], func=AF.Ln,
                         scale=128.0)
    nc.scalar.activation(out=rdz[op, :], in_=rdz[op, :], func=AF.Exp,
                         scale=-1.0)

    # --- accumulators, initialized with the center tap (w == 1)
    accw = accp.tile([128, n_out], F32, tag="accw")
    nc.vector.memset(accw[op, :], 1.0)
    acc3 = accp.tile([128, 3, n_out], F32, tag="acc3")
    nc.vector.tensor_copy(out=acc3[op, :, :], in_=ctr[op, 4:7, of])

    for dy in _dy_order():
        if dy == 0:
            sh = ctr
        else:
            sh = shp.tile([128, 7, n_in], F32, tag="sh", bufs=2)
            nc.sync.dma_start(
                out=sh[op, :, :],
                in_=in_t[RAD + dy:RAD + dy + TILE_OUT, 0:7, :])
        for adx in range(0, RAD + 1):
            d2 = dy * dy + adx * adx
            if d2 == 0 or d2 not in d2_set:
                continue
            rm = rmp.tile([128, n_out], F32, tag="rm", bufs=2)
            nc.gpsimd.tensor_scalar(out=rm[op, :], in0=rdz[op, :],
                                    scalar1=1.0 / math.sqrt(d2),
                                    scalar2=RM_CLAMP,
                                    op0=OP.mult, op1=OP.min)
            bias_ap = bias_tile[op, cls_idx[d2]:cls_idx[d2] + 1]
            for dx in ([0] if adx == 0 else [adx, -adx]):
                if (dy, dx) == (0, 0):
                    continue
                sf = slice(RAD + dx, RAD + dx + n_out)     # shifted columns
                # dot(nrm, nrm_shifted)
                pr = t3p.tile([128, 3, n_out], F32, tag="p3", bufs=2)
                nc.vector.tensor_tensor(out=pr[op, :, :],
                                        in0=ctr[op, 0:3, of],
                                        in1=sh[op, 0:3, sf], op=OP.mult)
                d = t1p.tile([128, n_out], F32, tag="s", bufs=2)
                nc.vector.tensor_tensor(out=d[op, :], in0=pr[op, 0, :],
                                        in1=pr[op, 1, :], op=OP.add)
                nc.vector.tensor_tensor(out=d[op, :], in0=d[op, :],
                                        in1=pr[op, 2, :], op=OP.add)
                lnu = t1p.tile([128, n_out], F32, tag="lnu", bufs=2)
                nc.scalar.activation(out=lnu[op, :], in_=d[op, :], func=AF.Ln)
                # |z_shift - z| * rm
                zd = t1p.tile([128, n_out], F32, tag="s", bufs=2)
                nc.gpsimd.tensor_tensor(out=zd[op, :], in0=sh[op, 3, sf],
                                        in1=ctr[op, 3, of], op=OP.subtract)
                nc.scalar.activation(out=zd[op, :], in_=zd[op, :], func=AF.Abs)
                nc.gpsimd.tensor_tensor(out=zd[op, :], in0=zd[op, :],
                                        in1=rm[op, :], op=OP.mult)
                nc.gpsimd.tensor_tensor(out=lnu[op, :], in0=lnu[op, :],
                                        in1=zd[op, :], op=OP.subtract)
                # w = exp(128*(ln(dot) - |zd|*rm) - d2/8)
                wt = t1p.tile([128, n_out], F32, tag="w", bufs=2)
                nc.scalar.activation(out=wt[op, :], in_=lnu[op, :],
                                     func=AF.Exp, scale=128.0, bias=bias_ap)
                nc.gpsimd.tensor_tensor(out=accw[op, :], in0=accw[op, :],
                                        in1=wt[op, :], op=OP.add)
                wc = t3p.tile([128, 3, n_out], F32, tag="p3", bufs=2)
                w_b = wt[op, None, :].to_broadcast((TILE_OUT, 3, n_out))
                nc.vector.tensor_tensor(out=wc[op, :, :],
                                        in0=sh[op, 4:7, sf],
                                        in1=w_b, op=OP.mult)
                nc.vector.tensor_tensor(out=acc3[op, :, :],
                                        in0=acc3[op, :, :],
                                        in1=wc[op, :, :], op=OP.add)

    # --- out = acc3 / accw
    nc.vector.reciprocal(out=accw[op, :], in_=accw[op, :])
    out3 = t3p.tile([128, 3, n_out], F32, tag="p3", bufs=2)
    rw_b = accw[op, None, :].to_broadcast((TILE_OUT, 3, n_out))
    nc.vector.tensor_tensor(out=out3[op, :, :], in0=acc3[op, :, :], in1=rw_b,
                            op=OP.mult)
    nc.sync.dma_start(out=dst_ap, in_=out3[op, :, :])


def _build(tensors, items, classes):
    """tensors: {name: (shape, kind)}; items: (in_name, col0, n_in, out_name,
    out_col0)."""
    nc = bacc.Bacc(None)
    handles = {}
    for name, (shape, kind) in tensors.items():
        handles[name] = nc.dram_tensor(name, list(shape), F32, kind=kind)
    cls_idx = {d2: k for k, (d2, _) in enumerate(classes)}
    # Preload the one act-table set containing Ln+Exp+Abs so the compiler's
    # per-activation table-load pass (first-containing-set policy) doesn't
    # thrash between the ln-only and exp-only sets on every tap.
    from concourse.hw_specs import get_activation_tables
    _tables = get_activation_tables(nc.m.arch)
    _need = {AF.Ln, AF.Exp, AF.Abs}
    _combined = next(i for i, (_, fs) in enumerate(_tables.items())
                     if _need <= fs)
    with tile.TileContext(nc) as tc:
        nc.scalar.add_instruction(mybir.InstLoadActFuncSet(
            act_func_set_id=_combined,
            name=nc.get_next_instruction_name(),
            engine=nc.scalar.engine,
            ins=[], outs=[]))
        with (
            tc.tile_pool(name="inp", bufs=1) as inp,
            tc.tile_pool(name="sh", bufs=1) as shp,
            tc.tile_pool(name="t3", bufs=1) as t3p,
            tc.tile_pool(name="t1", bufs=1) as t1p,
            tc.tile_pool(name="acc", bufs=1) as accp,
            tc.tile_pool(name="rm", bufs=1) as rmp,
            tc.tile_pool(name="bias", bufs=1) as biasp,
        ):
            bias_tile = biasp.tile([128, len(classes)], F32)
            for d2, k in cls_idx.items():
                nc.vector.memset(bias_tile[:, k:k + 1], -d2 / 8.0)
            pools = (inp, shp, t3p, t1p, accp, rmp)
            for in_name, col0, n_in, out_name, out_col0 in items:
                n_out = n_in - 2 * RAD
                src = handles[in_name][:, :, col0:col0 + n_in]
                src = src.rearrange("c h w -> h c w")
                dst = handles[out_name][:, :, out_col0:out_col0 + n_out]
                dst = dst.rearrange("c h w -> h c w")
                _emit_item(nc, pools, src, dst, n_in, classes, bias_tile,
                           cls_idx)
    nc.finalize()
    return nc


_CACHE = {}


def _get_full():
    if "full" not in _CACHE:
        tensors = {
            "xa": ((8, 128, W + 10), "ExternalInput"),
            "xb": ((8, 128, 490), "ExternalInput"),
            "ya": ((3, TILE_OUT, W), "ExternalOutput"),
            "yb": ((3, TILE_OUT, 480), "ExternalOutput"),
        }
        items = [
            ("xa", 0, 650, "ya", 0),
            ("xa", 640, 650, "ya", 640),
            ("xa", 1280, 650, "ya", 1280),
            ("xb", 0, 490, "yb", 0),
        ]
        _CACHE["full"] = _build(tensors, items, tap_classes())
    return _CACHE["full"]


def _get_mini(n_in=202, n_classes=None):
    key = ("mini", n_in, n_classes)
    classes = tap_classes()
    if n_classes is not None:
        classes = classes[:n_classes]
    if key not in _CACHE:
        n_out = n_in - 2 * RAD
        tensors = {
            "xm": ((8, 128, n_in), "ExternalInput"),
            "ym": ((3, TILE_OUT, n_out), "ExternalOutput"),
        }
        items = [("xm", 0, n_in, "ym", 0)]
        _CACHE[key] = _build(tensors, items, classes)
    return _CACHE[key], classes


def _make_planes(inp):
    """[H,W,8] -> zero-padded planes [8, H+10, W+10] in kernel plane order."""
    src = np.moveaxis(np.asarray(inp, dtype=np.float32), -1, 0)[PLANE_PERM]
    planes = np.zeros((8, src.shape[1] + 2 * RAD, src.shape[2] + 2 * RAD),
                      np.float32)
    planes[:, RAD:RAD + src.shape[1], RAD:RAD + src.shape[2]] = src
    return planes


LAST_RESULTS = None


def kernel(input, _trace=False):
    global LAST_RESULTS
    inp = np.asarray(input, dtype=np.float32)[0]          # [1080, 1920, 8]
    planes = _make_planes(inp)                            # [8, 1090, 1930]
    in_maps = []
    for i in range(8):
        xa = np.ascontiguousarray(planes[:, 118 * i:118 * i + 128, :])
        if i < 4:
            xb = planes[:, 944:1072, 480 * i:480 * i + 490]
        else:
            j = i - 4
            xb = planes[:, 962:1090, 480 * j:480 * j + 490]
        in_maps.append({"xa": xa, "xb": np.ascontiguousarray(xb)})
    nc = _get_full()
    res = run_bass_kernel_spmd(nc, in_maps, core_ids=list(range(8)),
                               trace=_trace)
    LAST_RESULTS = res
    out = np.empty((H, W, 3), np.float32)
    for i in range(8):
        out[118 * i:118 * i + 118] = np.moveaxis(res.results[i]["ya"], 0, -1)
    for i in range(8):
        yb = res.results[i]["yb"]
        if i < 4:
            out[944:1062, 480 * i:480 * i + 480] = np.moveaxis(yb, 0, -1)
        else:
            j = i - 4
            out[1062:1080, 480 * j:480 * j + 480] = (
                np.moveaxis(yb[:, 100:118, :], 0, -1))
    return out[None]


# revision 9
# speedup vs baseline: 1.8151x; 1.8151x over previous
"""Bilateral denoiser (11x11 window, sigma=2) on 8 Trainium2 NeuronCores.

Math (per output pixel p, tap offset t=(dy,dx), d2 = dy^2+dx^2):
    w_t = exp(128*ln(dot(nrm_p, nrm_{p+t})) - |z_{p+t}-z_p|*min(1/(dz_p*c), 1e4)
              - d2/8)
        = clip(dot,0,1)^128 * exp(-|dz|/max(dz*sqrt(d2), 1e-4)) * exp(-d2/8)
    out = sum_t w_t * col_{p+t} / sum_t w_t     (center tap has w=1)

Sharding: H=1080 rows -> 10 tile rows of exactly 108 output rows, each
computed from a 128-row input tile (+-10-row halo in the partition dim).
Every core gets tile-row i full-width plus one 480-wide strip of tile-rows
8/9, so all cores run an identical (SPMD) program on identical shapes.

Symmetry: dot(n_p, n_{p+t}) and |z_{p+t}-z_p| are symmetric in (p, p+t), so
each pair {t, -t} shares one dot/|dz| plane computed on 108+dy extended rows;
the +t member reads it through a partition-shifted DMA copy (engine access
patterns must start at partition 0 -- quadrant rule -- so shifts go via DMA).

Host side only pads/deinterleaves/slices (layout, no math); all arithmetic
runs on-device: DVE tensor ops + ScalarE Ln/Exp/Abs (one act table set).
"""
import math

import numpy as np

import concourse.bacc as bacc
import concourse.tile as tile
from concourse import mybir
from concourse.bass_utils import run_bass_kernel_spmd

F32 = mybir.dt.float32
AF = mybir.ActivationFunctionType
OP = mybir.AluOpType

RAD = 5
H, W = 1080, 1920
TILE_OUT = 108            # output rows per 128-partition tile (2*RAD halo x2)
VPAD = 2 * RAD            # vertical halo rows above/below each tile
RM_CLAMP = 1.0 / (128.0 * 1e-4)

# plane order inside the 8-plane tensors
# 0:3 = normal(xyz), 3 = z, 4:7 = color(rgb), 7 = dz
PLANE_PERM = [3, 4, 5, 6, 0, 1, 2, 7]  # from input channel order


def tap_classes(rad=RAD):
    cls = {}
    for dy in range(-rad, rad + 1):
        for dx in range(-rad, rad + 1):
            if dy == 0 and dx == 0:
                continue
            cls.setdefault(dy * dy + dx * dx, []).append((dy, dx))
    return sorted(cls.items())


def _pairs(classes):
    """Pairs {(dy,dx), (-dy,-dx)} grouped by dy >= 0; rep has dy>0 or
    (dy==0 and dx>0). Returns {dy: [dx,...]} honoring the class subset."""
    tap_set = {t for _, taps in classes for t in taps}
    groups = {}
    for dy in range(0, RAD + 1):
        dxs = []
        for dx in range(-RAD, RAD + 1):
            if dy == 0 and dx <= 0:
                continue
            if (dy, dx) in tap_set:
                dxs.append(dx)
        if dxs:
            # order by |dx| so +-dx neighbors share the rm plane
            dxs.sort(key=lambda d: (abs(d), -d))
            groups[dy] = dxs
    return groups


def _emit_member(nc, pools, ctx, lnu_ap, azd_ap, col_ap, rm_ap, bias_ap):
    """Accumulate one tap given its aligned ln(dot), |dz|, color APs."""
    t1p, t3p = pools
    op, n_out, accw, acc3 = ctx
    t1 = t1p.tile([128, n_out], F32, tag="s", bufs=3)
    nc.vector.tensor_tensor(out=t1[op, :], in0=azd_ap, in1=rm_ap, op=OP.mult)
    nc.vector.tensor_tensor(out=t1[op, :], in0=lnu_ap, in1=t1[op, :],
                            op=OP.subtract)
    wt = t1p.tile([128, n_out], F32, tag="w", bufs=3)
    nc.scalar.activation(out=wt[op, :], in_=t1[op, :], func=AF.Exp,
                         scale=128.0, bias=bias_ap)
    nc.vector.tensor_tensor(out=accw[op, :], in0=accw[op, :], in1=wt[op, :],
                            op=OP.add)
    wc = t3p.tile([128, 3, n_out], F32, tag="p3", bufs=3)
    w_b = wt[op, None, :].to_broadcast((TILE_OUT, 3, n_out))
    nc.vector.tensor_tensor(out=wc[op, :, :], in0=col_ap, in1=w_b, op=OP.mult)
    nc.vector.tensor_tensor(out=acc3[op, :, :], in0=acc3[op, :, :],
                            in1=wc[op, :, :], op=OP.add)


def _emit_item(nc, pools, src_ap, dst_ap, n_in, classes, bias_tile, cls_idx):
    """One work item: input planes [8,128,n_in] -> output [108,3,n_in-10]."""
    n_out = n_in - 2 * RAD
    inp, shp, t3p, t1p, accp, rmp = pools
    op = slice(0, TILE_OUT)            # output rows = in_t rows [10,118)
    of = slice(RAD, RAD + n_out)       # center free columns within n_in

    pair_groups = _pairs(classes)

    in_t = inp.tile([128, 8, n_in], F32, tag="in", bufs=1)
    nc.sync.dma_start(out=in_t[:, :, :], in_=src_ap)

    # --- normalize normals in place: n *= (max(|n|^2,1e-20))^-0.5
    pr3 = t3p.tile([128, 3, n_in], F32, tag="p3", bufs=3)
    nc.vector.tensor_tensor(out=pr3[:, :, :], in0=in_t[:, 0:3, :],
                            in1=in_t[:, 0:3, :], op=OP.mult)
    nn = t1p.tile([128, n_in], F32, tag="s", bufs=3)
    nc.vector.tensor_tensor(out=nn[:, :], in0=pr3[:, 0, :], in1=pr3[:, 1, :],
                            op=OP.add)
    nc.vector.tensor_tensor(out=nn[:, :], in0=nn[:, :], in1=pr3[:, 2, :],
                            op=OP.add)
    nc.vector.tensor_scalar_max(out=nn[:, :], in0=nn[:, :], scalar1=1e-20)
    nc.scalar.activation(out=nn[:, :], in_=nn[:, :], func=AF.Ln)
    nc.scalar.activation(out=nn[:, :], in_=nn[:, :], func=AF.Exp, scale=-0.5)
    for c in range(3):
        nc.vector.tensor_tensor(out=in_t[:, c, :], in0=in_t[:, c, :],
                                in1=nn[:, :], op=OP.mult)

    # --- partition-aligned center copy: ctr[p] = in_t[p+10], 118 rows so the
    # extended dot rows (108+dy <= 113) stay in range
    ctr = shp.tile([128, 8, n_in], F32, tag="ctr", bufs=1)
    nc.sync.dma_start(out=ctr[0:118, :, :], in_=in_t[VPAD:128, :, :])

    # --- rdz = 1/(128*dz) on center columns (inf when dz==0; min-clamped)
    rdz = t1p.tile([128, n_out], F32, tag="rdz", bufs=2)
    nc.scalar.activation(out=rdz[op, :], in_=ctr[op, 7, of], func=AF.Ln,
                         scale=128.0)
    nc.scalar.activation(out=rdz[op, :], in_=rdz[op, :], func=AF.Exp,
                         scale=-1.0)

    # --- accumulators, initialized with the center tap (w == 1)
    accw = accp.tile([128, n_out], F32, tag="accw")
    nc.vector.memset(accw[op, :], 1.0)
    acc3 = accp.tile([128, 3, n_out], F32, tag="acc3")
    nc.vector.tensor_copy(out=acc3[op, :, :], in_=ctr[op, 4:7, of])

    mctx = (op, n_out, accw, acc3)

    for dy, dxs in pair_groups.items():
        u = TILE_OUT + dy              # extended dot rows
        eop = slice(0, u)
        if dy == 0:
            sh_m = ctr                 # member reads resolve against ctr
            sh_p = None
        else:
            # sh_m[p] = in_t[p+10-dy]: normals+z+colors for the -t member and
            # the shared dot/|dz| planes
            sh_m = shp.tile([128, 8, n_in], F32, tag="sh_m", bufs=2)
            nc.sync.dma_start(out=sh_m[0:u, :, :],
                              in_=in_t[VPAD - dy:VPAD - dy + u, :, :])
            # sh_p[p] = in_t[p+10+dy]: colors for the +t member
            sh_p = shp.tile([128, 3, n_in], F32, tag="sh_p", bufs=2)
            nc.sync.dma_start(out=sh_p[op, :, :],
                              in_=in_t[VPAD + dy:VPAD + dy + TILE_OUT,
                                       4:7, :])
        rm = None
        last_adx = None
        for dx in dxs:
            d2 = dy * dy + dx * dx
            if abs(dx) != last_adx:
                last_adx = abs(dx)
                rm = rmp.tile([128, n_out], F32, tag="rm", bufs=2)
                nc.vector.tensor_scalar(out=rm[op, :], in0=rdz[op, :],
                                        scalar1=1.0 / math.sqrt(d2),
                                        scalar2=RM_CLAMP,
                                        op0=OP.mult, op1=OP.min)
            bias_ap = bias_tile[op, cls_idx[d2]:cls_idx[d2] + 1]
            lo = max(0, -dx)
            hi = n_in - max(0, dx)
            # shared planes: P2[.,0,x] = dot(n(r), n(r+t)) at base row
            # r = p+10-dy;  P2[.,1,x] = |z(r+t) - z(r)|
            pr = t3p.tile([128, 3, n_in], F32, tag="p3", bufs=3)
            nc.vector.tensor_tensor(out=pr[eop, :, lo:hi],
                                    in0=sh_m[eop, 0:3, lo:hi],
                                    in1=ctr[eop, 0:3, lo + dx:hi + dx],
                                    op=OP.mult)
            p2 = shp.tile([128, 2, n_in], F32, tag="p2", bufs=2)
            nc.vector.tensor_tensor(out=p2[eop, 0, lo:hi],
                                    in0=pr[eop, 0, lo:hi],
                                    in1=pr[eop, 1, lo:hi], op=OP.add)
            nc.vector.tensor_tensor(out=p2[eop, 0, lo:hi],
                                    in0=p2[eop, 0, lo:hi],
                                    in1=pr[eop, 2, lo:hi], op=OP.add)
            nc.vector.tensor_tensor(out=p2[eop, 1, lo:hi],
                                    in0=ctr[eop, 3, lo + dx:hi + dx],
                                    in1=sh_m[eop, 3, lo:hi], op=OP.subtract)
            nc.scalar.activation(out=p2[eop, 0, lo:hi], in_=p2[eop, 0, lo:hi],
                                 func=AF.Ln)
            nc.scalar.activation(out=p2[eop, 1, lo:hi], in_=p2[eop, 1, lo:hi],
                                 func=AF.Abs)
            # member -t = (-dy,-dx): aligned rows, columns shifted by -dx
            sm = slice(RAD - dx, RAD - dx + n_out)
            col_m = (ctr[op, 4:7, sm] if dy == 0 else sh_m[op, 4:7, sm])
            _emit_member(nc, (t1p, t3p), mctx,
                         p2[op, 0, sm], p2[op, 1, sm], col_m,
                         rm[op, :], bias_ap)
            # member +t = (dy,dx): rows shifted by +dy (DMA copy when dy>0)
            sfp = slice(RAD + dx, RAD + dx + n_out)
            if dy == 0:
                lnu_p, azd_p = p2[op, 0, of], p2[op, 1, of]
                col_p = ctr[op, 4:7, sfp]
            else:
                p2p = shp.tile([128, 2, n_out], F32, tag="p2s", bufs=2)
                nc.sync.dma_start(out=p2p[op, :, :],
                                  in_=p2[dy:dy + TILE_OUT, :, of])
                lnu_p, azd_p = p2p[op, 0, :], p2p[op, 1, :]
                col_p = sh_p[op, 0:3, sfp]
            _emit_member(nc, (t1p, t3p), mctx,
                         lnu_p, azd_p, col_p, rm[op, :], bias_ap)

    # --- out = acc3 / accw
    nc.vector.reciprocal(out=accw[op, :], in_=accw[op, :])
    out3 = t3p.tile([128, 3, n_out], F32, tag="p3", bufs=3)
    rw_b = accw[op, None, :].to_broadcast((TILE_OUT, 3, n_out))
    nc.vector.tensor_tensor(out=out3[op, :, :], in0=acc3[op, :, :], in1=rw_b,
                            op=OP.mult)
    nc.sync.dma_start(out=dst_ap, in_=out3[op, :, :])


def _build(tensors, items, classes):
    """tensors: {name: (shape, kind)}; items: (in_name, col0, n_in, out_name,
    out_col0)."""
    nc = bacc.Bacc(None)
    handles = {}
    for name, (shape, kind) in tensors.items():
        handles[name] = nc.dram_tensor(name, list(shape), F32, kind=kind)
    cls_idx = {d2: k for k, (d2, _) in enumerate(classes)}
    # Preload the one act-table set containing Ln+Exp+Abs so the compiler's
    # per-activation table-load pass (first-containing-set policy) doesn't
    # thrash between the ln-only and exp-only sets on every tap.
    from concourse.hw_specs import get_activation_tables
    _tables = get_activation_tables(nc.m.arch)
    _need = {AF.Ln, AF.Exp, AF.Abs}
    _combined = next(i for i, (_, fs) in enumerate(_tables.items())
                     if _need <= fs)
    with tile.TileContext(nc) as tc:
        nc.scalar.add_instruction(mybir.InstLoadActFuncSet(
            act_func_set_id=_combined,
            name=nc.get_next_instruction_name(),
            engine=nc.scalar.engine,
            ins=[], outs=[]))
        with (
            tc.tile_pool(name="inp", bufs=1) as inp,
            tc.tile_pool(name="sh", bufs=1) as shp,
            tc.tile_pool(name="t3", bufs=1) as t3p,
            tc.tile_pool(name="t1", bufs=1) as t1p,
            tc.tile_pool(name="acc", bufs=1) as accp,
            tc.tile_pool(name="rm", bufs=1) as rmp,
            tc.tile_pool(name="bias", bufs=1) as biasp,
        ):
            bias_tile = biasp.tile([128, len(classes)], F32)
            for d2, k in cls_idx.items():
                nc.vector.memset(bias_tile[:, k:k + 1], -d2 / 8.0)
            pools = (inp, shp, t3p, t1p, accp, rmp)
            for in_name, col0, n_in, out_name, out_col0 in items:
                n_out = n_in - 2 * RAD
                src = handles[in_name][:, :, col0:col0 + n_in]
                src = src.rearrange("c h w -> h c w")
                dst = handles[out_name][:, :, out_col0:out_col0 + n_out]
                dst = dst.rearrange("c h w -> h c w")
                _emit_item(nc, pools, src, dst, n_in, classes, bias_tile,
                           cls_idx)
    nc.finalize()
    return nc


_CACHE = {}


def _get_full():
    if "full" not in _CACHE:
        tensors = {
            "xa": ((8, 128, W + 10), "ExternalInput"),
            "xb": ((8, 128, 490), "ExternalInput"),
            "ya": ((3, TILE_OUT, W), "ExternalOutput"),
            "yb": ((3, TILE_OUT, 480), "ExternalOutput"),
        }
        items = [
            ("xa", 0, 650, "ya", 0),
            ("xa", 640, 650, "ya", 640),
            ("xa", 1280, 650, "ya", 1280),
            ("xb", 0, 490, "yb", 0),
        ]
        _CACHE["full"] = _build(tensors, items, tap_classes())
    return _CACHE["full"]


def _get_mini(n_in=202, n_classes=None):
    key = ("mini", n_in, n_classes)
    classes = tap_classes()
    if n_classes is not None:
        classes = classes[:n_classes]
    if key not in _CACHE:
        n_out = n_in - 2 * RAD
        tensors = {
            "xm": ((8, 128, n_in), "ExternalInput"),
            "ym": ((3, TILE_OUT, n_out), "ExternalOutput"),
        }
        items = [("xm", 0, n_in, "ym", 0)]
        _CACHE[key] = _build(tensors, items, classes)
    return _CACHE[key], classes


def _make_planes(inp):
    """[H,W,8] -> padded planes [8, H+2*VPAD, W+2*RAD], kernel plane order."""
    src = np.moveaxis(np.asarray(inp, dtype=np.float32), -1, 0)[PLANE_PERM]
    planes = np.zeros((8, src.shape[1] + 2 * VPAD, src.shape[2] + 2 * RAD),
                      np.float32)
    planes[:, VPAD:VPAD + src.shape[1], RAD:RAD + src.shape[2]] = src
    return planes


LAST_RESULTS = None


def kernel(input, _trace=False):
    global LAST_RESULTS
    inp = np.asarray(input, dtype=np.float32)[0]          # [1080, 1920, 8]
    planes = _make_planes(inp)                            # [8, 1100, 1930]
    T = TILE_OUT
    in_maps = []
    for i in range(8):
        xa = np.ascontiguousarray(planes[:, T * i:T * i + 128, :])
        if i < 4:
            # tile-row 8: output rows [864, 972)
            xb = planes[:, 8 * T:8 * T + 128, 480 * i:480 * i + 490]
        else:
            # tile-row 9: output rows [972, 1080)
            j = i - 4
            xb = planes[:, 9 * T:9 * T + 128, 480 * j:480 * j + 490]
        in_maps.append({"xa": xa, "xb": np.ascontiguousarray(xb)})
    nc = _get_full()
    res = run_bass_kernel_spmd(nc, in_maps, core_ids=list(range(8)),
                               trace=_trace)
    LAST_RESULTS = res
    out = np.empty((H, W, 3), np.float32)
    for i in range(8):
        out[T * i:T * i + T] = np.moveaxis(res.results[i]["ya"], 0, -1)
    for i in range(8):
        yb = np.moveaxis(res.results[i]["yb"], 0, -1)
        if i < 4:
            out[8 * T:9 * T, 480 * i:480 * i + 480] = yb
        else:
            j = i - 4
            out[9 * T:10 * T, 480 * j:480 * j + 480] = yb
    return out[None]


# revision 10
# speedup vs baseline: 1.8487x; 1.0185x over previous
"""Bilateral denoiser (11x11 window, sigma=2) on 8 Trainium2 NeuronCores.

Math (per output pixel p, tap offset t=(dy,dx), d2 = dy^2+dx^2):
    w_t = exp(128*ln(dot(nrm_p, nrm_{p+t})) - |z_{p+t}-z_p|*min(1/(dz_p*c), 1e4)
              - d2/8)
        = clip(dot,0,1)^128 * exp(-|dz|/max(dz*sqrt(d2), 1e-4)) * exp(-d2/8)
    out = sum_t w_t * col_{p+t} / sum_t w_t     (center tap has w=1)

Sharding: H=1080 rows -> 10 tile rows of exactly 108 output rows, each
computed from a 128-row input tile (+-10-row halo in the partition dim).
Every core gets tile-row i full-width plus one 480-wide strip of tile-rows
8/9, so all cores run an identical (SPMD) program on identical shapes.

Symmetry: dot(n_p, n_{p+t}) and |z_{p+t}-z_p| are symmetric in (p, p+t), so
each pair {t, -t} shares one dot/|dz| plane computed on 108+dy extended rows;
the +t member reads it through a partition-shifted DMA copy (engine access
patterns must start at partition 0 -- quadrant rule -- so shifts go via DMA).

Host side only pads/deinterleaves/slices (layout, no math); all arithmetic
runs on-device: DVE tensor ops + ScalarE Ln/Exp/Abs (one act table set).
"""
import math

import numpy as np

import concourse.bacc as bacc
import concourse.tile as tile
from concourse import mybir
from concourse.bass_utils import run_bass_kernel_spmd

F32 = mybir.dt.float32
AF = mybir.ActivationFunctionType
OP = mybir.AluOpType

RAD = 5
H, W = 1080, 1920
TILE_OUT = 108            # output rows per 128-partition tile (2*RAD halo x2)
VPAD = 2 * RAD            # vertical halo rows above/below each tile
RM_CLAMP = 1.0 / (128.0 * 1e-4)

# plane order inside the 8-plane tensors
# 0:3 = normal(xyz), 3 = z, 4:7 = color(rgb), 7 = dz
PLANE_PERM = [3, 4, 5, 6, 0, 1, 2, 7]  # from input channel order


def tap_classes(rad=RAD):
    cls = {}
    for dy in range(-rad, rad + 1):
        for dx in range(-rad, rad + 1):
            if dy == 0 and dx == 0:
                continue
            cls.setdefault(dy * dy + dx * dx, []).append((dy, dx))
    return sorted(cls.items())


def _pairs(classes):
    """Pairs {(dy,dx), (-dy,-dx)} grouped by dy >= 0; rep has dy>0 or
    (dy==0 and dx>0). Returns {dy: [dx,...]} honoring the class subset."""
    tap_set = {t for _, taps in classes for t in taps}
    groups = {}
    for dy in range(0, RAD + 1):
        dxs = []
        for dx in range(-RAD, RAD + 1):
            if dy == 0 and dx <= 0:
                continue
            if (dy, dx) in tap_set:
                dxs.append(dx)
        if dxs:
            # order by |dx| so +-dx neighbors share the rm plane
            dxs.sort(key=lambda d: (abs(d), -d))
            groups[dy] = dxs
    return groups


def _emit_member(nc, pools, ctx, lnu_ap, azd_ap, col_ap, rm_ap, bias_ap):
    """Accumulate one tap given its aligned ln(dot), |dz|, color APs."""
    t1p, t3p = pools
    op, n_out, accw, acc3 = ctx
    t1 = t1p.tile([128, n_out], F32, tag="s", bufs=3)
    nc.vector.tensor_tensor(out=t1[op, :], in0=azd_ap, in1=rm_ap, op=OP.mult)
    nc.vector.tensor_tensor(out=t1[op, :], in0=lnu_ap, in1=t1[op, :],
                            op=OP.subtract)
    wt = t1p.tile([128, n_out], F32, tag="w", bufs=3)
    nc.scalar.activation(out=wt[op, :], in_=t1[op, :], func=AF.Exp,
                         scale=128.0, bias=bias_ap)
    nc.vector.tensor_tensor(out=accw[op, :], in0=accw[op, :], in1=wt[op, :],
                            op=OP.add)
    wc = t3p.tile([128, 3, n_out], F32, tag="p3", bufs=2)
    w_b = wt[op, None, :].to_broadcast((TILE_OUT, 3, n_out))
    nc.vector.tensor_tensor(out=wc[op, :, :], in0=col_ap, in1=w_b, op=OP.mult)
    nc.vector.tensor_tensor(out=acc3[op, :, :], in0=acc3[op, :, :],
                            in1=wc[op, :, :], op=OP.add)


def _emit_item(nc, pools, src_ap, dst_ap, n_in, classes, bias_tile, cls_idx):
    """One work item: input planes [8,128,n_in] -> output [108,3,n_in-10]."""
    n_out = n_in - 2 * RAD
    inp, shp, t3p, t1p, accp, rmp = pools
    op = slice(0, TILE_OUT)            # output rows = in_t rows [10,118)
    of = slice(RAD, RAD + n_out)       # center free columns within n_in

    pair_groups = _pairs(classes)

    in_t = inp.tile([128, 8, n_in], F32, tag="in", bufs=2)
    nc.sync.dma_start(out=in_t[:, :, :], in_=src_ap)

    # --- normalize normals in place: n *= (max(|n|^2,1e-20))^-0.5
    pr3 = t3p.tile([128, 3, n_in], F32, tag="p3", bufs=2)
    nc.vector.tensor_tensor(out=pr3[:, :, :], in0=in_t[:, 0:3, :],
                            in1=in_t[:, 0:3, :], op=OP.mult)
    nn = t1p.tile([128, n_in], F32, tag="s", bufs=3)
    nc.vector.tensor_tensor(out=nn[:, :], in0=pr3[:, 0, :], in1=pr3[:, 1, :],
                            op=OP.add)
    nc.vector.tensor_tensor(out=nn[:, :], in0=nn[:, :], in1=pr3[:, 2, :],
                            op=OP.add)
    nc.vector.tensor_scalar_max(out=nn[:, :], in0=nn[:, :], scalar1=1e-20)
    nc.scalar.activation(out=nn[:, :], in_=nn[:, :], func=AF.Ln)
    nc.scalar.activation(out=nn[:, :], in_=nn[:, :], func=AF.Exp, scale=-0.5)
    for c in range(3):
        nc.vector.tensor_tensor(out=in_t[:, c, :], in0=in_t[:, c, :],
                                in1=nn[:, :], op=OP.mult)

    # --- partition-aligned center copy: ctr[p] = in_t[p+10], 118 rows so the
    # extended dot rows (108+dy <= 113) stay in range
    ctr = shp.tile([128, 8, n_in], F32, tag="ctr", bufs=1)
    nc.sync.dma_start(out=ctr[0:118, :, :], in_=in_t[VPAD:128, :, :])

    # --- rdz = 1/(128*dz) on center columns (inf when dz==0; min-clamped)
    rdz = t1p.tile([128, n_out], F32, tag="rdz", bufs=2)
    nc.scalar.activation(out=rdz[op, :], in_=ctr[op, 7, of], func=AF.Ln,
                         scale=128.0)
    nc.scalar.activation(out=rdz[op, :], in_=rdz[op, :], func=AF.Exp,
                         scale=-1.0)

    # --- accumulators, initialized with the center tap (w == 1)
    accw = accp.tile([128, n_out], F32, tag="accw")
    nc.vector.memset(accw[op, :], 1.0)
    acc3 = accp.tile([128, 3, n_out], F32, tag="acc3")
    nc.vector.tensor_copy(out=acc3[op, :, :], in_=ctr[op, 4:7, of])

    mctx = (op, n_out, accw, acc3)

    for dy, dxs in pair_groups.items():
        u = TILE_OUT + dy              # extended dot rows
        eop = slice(0, u)
        if dy == 0:
            sh_m = ctr                 # member reads resolve against ctr
            sh_p = None
        else:
            # sh_m[p] = in_t[p+10-dy]: normals+z+colors for the -t member and
            # the shared dot/|dz| planes
            sh_m = shp.tile([128, 7, n_in], F32, tag="sh_m", bufs=2)
            nc.sync.dma_start(out=sh_m[0:u, :, :],
                              in_=in_t[VPAD - dy:VPAD - dy + u, 0:7, :])
            # sh_p[p] = in_t[p+10+dy]: colors for the +t member
            sh_p = shp.tile([128, 3, n_in], F32, tag="sh_p", bufs=2)
            nc.sync.dma_start(out=sh_p[op, :, :],
                              in_=in_t[VPAD + dy:VPAD + dy + TILE_OUT,
                                       4:7, :])
        rm = None
        last_adx = None
        for dx in dxs:
            d2 = dy * dy + dx * dx
            if abs(dx) != last_adx:
                last_adx = abs(dx)
                rm = rmp.tile([128, n_out], F32, tag="rm", bufs=2)
                nc.vector.tensor_scalar(out=rm[op, :], in0=rdz[op, :],
                                        scalar1=1.0 / math.sqrt(d2),
                                        scalar2=RM_CLAMP,
                                        op0=OP.mult, op1=OP.min)
            bias_ap = bias_tile[op, cls_idx[d2]:cls_idx[d2] + 1]
            lo = max(0, -dx)
            hi = n_in - max(0, dx)
            # shared planes: P2[.,0,x] = dot(n(r), n(r+t)) at base row
            # r = p+10-dy;  P2[.,1,x] = |z(r+t) - z(r)|
            pr = t3p.tile([128, 3, n_in], F32, tag="p3", bufs=2)
            nc.vector.tensor_tensor(out=pr[eop, :, lo:hi],
                                    in0=sh_m[eop, 0:3, lo:hi],
                                    in1=ctr[eop, 0:3, lo + dx:hi + dx],
                                    op=OP.mult)
            p2 = shp.tile([128, 2, n_in], F32, tag="p2", bufs=2)
            nc.vector.tensor_tensor(out=p2[eop, 0, lo:hi],
                                    in0=pr[eop, 0, lo:hi],
                                    in1=pr[eop, 1, lo:hi], op=OP.add)
            nc.vector.tensor_tensor(out=p2[eop, 0, lo:hi],
                                    in0=p2[eop, 0, lo:hi],
                                    in1=pr[eop, 2, lo:hi], op=OP.add)
            nc.vector.tensor_tensor(out=p2[eop, 1, lo:hi],
                                    in0=ctr[eop, 3, lo + dx:hi + dx],
                                    in1=sh_m[eop, 3, lo:hi], op=OP.subtract)
            nc.scalar.activation(out=p2[eop, 0, lo:hi], in_=p2[eop, 0, lo:hi],
                                 func=AF.Ln)
            nc.scalar.activation(out=p2[eop, 1, lo:hi], in_=p2[eop, 1, lo:hi],
                                 func=AF.Abs)
            # member -t = (-dy,-dx): aligned rows, columns shifted by -dx
            sm = slice(RAD - dx, RAD - dx + n_out)
            col_m = (ctr[op, 4:7, sm] if dy == 0 else sh_m[op, 4:7, sm])
            _emit_member(nc, (t1p, t3p), mctx,
                         p2[op, 0, sm], p2[op, 1, sm], col_m,
                         rm[op, :], bias_ap)
            # member +t = (dy,dx): rows shifted by +dy (DMA copy when dy>0)
            sfp = slice(RAD + dx, RAD + dx + n_out)
            if dy == 0:
                lnu_p, azd_p = p2[op, 0, of], p2[op, 1, of]
                col_p = ctr[op, 4:7, sfp]
            else:
                p2p = shp.tile([128, 2, n_out], F32, tag="p2s", bufs=2)
                nc.sync.dma_start(out=p2p[op, :, :],
                                  in_=p2[dy:dy + TILE_OUT, :, of])
                lnu_p, azd_p = p2p[op, 0, :], p2p[op, 1, :]
                col_p = sh_p[op, 0:3, sfp]
            _emit_member(nc, (t1p, t3p), mctx,
                         lnu_p, azd_p, col_p, rm[op, :], bias_ap)

    # --- out = acc3 / accw
    nc.vector.reciprocal(out=accw[op, :], in_=accw[op, :])
    out3 = t3p.tile([128, 3, n_out], F32, tag="p3", bufs=2)
    rw_b = accw[op, None, :].to_broadcast((TILE_OUT, 3, n_out))
    nc.vector.tensor_tensor(out=out3[op, :, :], in0=acc3[op, :, :], in1=rw_b,
                            op=OP.mult)
    nc.sync.dma_start(out=dst_ap, in_=out3[op, :, :])


def _build(tensors, items, classes):
    """tensors: {name: (shape, kind)}; items: (in_name, col0, n_in, out_name,
    out_col0)."""
    nc = bacc.Bacc(None)
    handles = {}
    for name, (shape, kind) in tensors.items():
        handles[name] = nc.dram_tensor(name, list(shape), F32, kind=kind)
    cls_idx = {d2: k for k, (d2, _) in enumerate(classes)}
    # Preload the one act-table set containing Ln+Exp+Abs so the compiler's
    # per-activation table-load pass (first-containing-set policy) doesn't
    # thrash between the ln-only and exp-only sets on every tap.
    from concourse.hw_specs import get_activation_tables
    _tables = get_activation_tables(nc.m.arch)
    _need = {AF.Ln, AF.Exp, AF.Abs}
    _combined = next(i for i, (_, fs) in enumerate(_tables.items())
                     if _need <= fs)
    with tile.TileContext(nc) as tc:
        nc.scalar.add_instruction(mybir.InstLoadActFuncSet(
            act_func_set_id=_combined,
            name=nc.get_next_instruction_name(),
            engine=nc.scalar.engine,
            ins=[], outs=[]))
        with (
            tc.tile_pool(name="inp", bufs=1) as inp,
            tc.tile_pool(name="sh", bufs=1) as shp,
            tc.tile_pool(name="t3", bufs=1) as t3p,
            tc.tile_pool(name="t1", bufs=1) as t1p,
            tc.tile_pool(name="acc", bufs=1) as accp,
            tc.tile_pool(name="rm", bufs=1) as rmp,
            tc.tile_pool(name="bias", bufs=1) as biasp,
        ):
            bias_tile = biasp.tile([128, len(classes)], F32)
            for d2, k in cls_idx.items():
                nc.vector.memset(bias_tile[:, k:k + 1], -d2 / 8.0)
            pools = (inp, shp, t3p, t1p, accp, rmp)
            for in_name, col0, n_in, out_name, out_col0 in items:
                n_out = n_in - 2 * RAD
                src = handles[in_name][:, :, col0:col0 + n_in]
                src = src.rearrange("c h w -> h c w")
                dst = handles[out_name][:, :, out_col0:out_col0 + n_out]
                dst = dst.rearrange("c h w -> h c w")
                _emit_item(nc, pools, src, dst, n_in, classes, bias_tile,
                           cls_idx)
    nc.finalize()
    return nc


_CACHE = {}


def _get_full():
    if "full" not in _CACHE:
        tensors = {
            "xa": ((8, 128, W + 10), "ExternalInput"),
            "xb": ((8, 128, 490), "ExternalInput"),
            "ya": ((3, TILE_OUT, W), "ExternalOutput"),
            "yb": ((3, TILE_OUT, 480), "ExternalOutput"),
        }
        items = [
            ("xa", 0, 650, "ya", 0),
            ("xa", 640, 650, "ya", 640),
            ("xa", 1280, 650, "ya", 1280),
            ("xb", 0, 490, "yb", 0),
        ]
        _CACHE["full"] = _build(tensors, items, tap_classes())
    return _CACHE["full"]


def _get_mini(n_in=202, n_classes=None):
    key = ("mini", n_in, n_classes)
    classes = tap_classes()
    if n_classes is not None:
        classes = classes[:n_classes]
    if key not in _CACHE:
        n_out = n_in - 2 * RAD
        tensors = {
            "xm": ((8, 128, n_in), "ExternalInput"),
            "ym": ((3, TILE_OUT, n_out), "ExternalOutput"),
        }
        items = [("xm", 0, n_in, "ym", 0)]
        _CACHE[key] = _build(tensors, items, classes)
    return _CACHE[key], classes


def _make_planes(inp):
    """[H,W,8] -> padded planes [8, H+2*VPAD, W+2*RAD], kernel plane order."""
    src = np.moveaxis(np.asarray(inp, dtype=np.float32), -1, 0)[PLANE_PERM]
    planes = np.zeros((8, src.shape[1] + 2 * VPAD, src.shape[2] + 2 * RAD),
                      np.float32)
    planes[:, VPAD:VPAD + src.shape[1], RAD:RAD + src.shape[2]] = src
    return planes


LAST_RESULTS = None


def kernel(input, _trace=False):
    global LAST_RESULTS
    inp = np.asarray(input, dtype=np.float32)[0]          # [1080, 1920, 8]
    planes = _make_planes(inp)                            # [8, 1100, 1930]
    T = TILE_OUT
    in_maps = []
    for i in range(8):
        xa = np.ascontiguousarray(planes[:, T * i:T * i + 128, :])
        if i < 4:
            # tile-row 8: output rows [864, 972)
            xb = planes[:, 8 * T:8 * T + 128, 480 * i:480 * i + 490]
        else:
            # tile-row 9: output rows [972, 1080)
            j = i - 4
            xb = planes[:, 9 * T:9 * T + 128, 480 * j:480 * j + 490]
        in_maps.append({"xa": xa, "xb": np.ascontiguousarray(xb)})
    nc = _get_full()
    res = run_bass_kernel_spmd(nc, in_maps, core_ids=list(range(8)),
                               trace=_trace)
    LAST_RESULTS = res
    out = np.empty((H, W, 3), np.float32)
    for i in range(8):
        out[T * i:T * i + T] = np.moveaxis(res.results[i]["ya"], 0, -1)
    for i in range(8):
        yb = np.moveaxis(res.results[i]["yb"], 0, -1)
        if i < 4:
            out[8 * T:9 * T, 480 * i:480 * i + 480] = yb
        else:
            j = i - 4
            out[9 * T:10 * T, 480 * j:480 * j + 480] = yb
    return out[None]
